# revision 1
# baseline (speedup 1.0000x reference)
"""LocallyConnected3D (valid, stride 1) as a TRN2 Bass kernel on 8 NeuronCores.

Math: out[b,l,f] = sum_p patch[b,l,p] * K[l,p,f] + bias[l,f]
  with B=4, L=27000 output locations, P=216 receptive field, F=16 filters.

The (L,P,F) kernel tensor dominates HBM traffic: each element is used exactly
B=4 times, so the problem is HBM-bandwidth-bound.  All operands stream in
bf16 (halves bytes vs fp32; adds ~4e-3 rel err, well under the 2e-2 gate):

- Shard L across the 8 cores (3375 locations each, padded to 3456 = 108
  groups of 32 locations).
- Per group g (32 locations), stream the kernel slab through the PE array as
  the *moving* operand:   psum1[m=(l',b), n=(l,f)] = sum_p patch[b,l',p]*K[l,p,f]
  via two accumulating bf16 matmuls (contraction split p=[0,128) and
  p=[128,216) plus one all-ones lhsT row carrying bias[l,f]).  Only the
  block-diagonal l'==l entries are wanted; PE redundancy is free because the
  kernel byte stream is the bottleneck.
- DVE multiplies psum1 by a constant block-diagonal mask (-> bf16),
- MM2 with a per-group selection matrix sel_g [128,128] sums over l' AND
  accumulates 27 consecutive groups into one PSUM bank at rows 4*(g%27)+b,
  so the PSUM->SBUF copy + output DMA run once per 27 groups.
- MM2 for group g is emitted 2 groups late so the PE never stalls waiting
  for the DVE mask-multiply.

DMA-efficiency notes (measured on this setup):
- Transfers whose partition count is not a multiple of 16 cannot spray
  across the 16 SDMA engines and run ~14x slower (33 vs 465+ GB/s), so the
  89-row second contraction chunk is padded to 96 rows and the 108-row
  output stage to 128 rows.
- The kernel and patch streams for each super-tile are packed side by side
  in one DRAM tensor so a super is 2 big DMAs instead of 4.

Host-side numpy does the im2col patch extraction and packs kernel/patches
into the exact SBUF tile layouts (index shuffling + bf16 cast only - all
FLOPs happen on device).
"""

from collections import deque
from contextlib import ExitStack

import ml_dtypes
import numpy as np

import concourse.bacc as bacc
import concourse.mybir as mybir
import concourse.tile as tile
from concourse import bass_utils

F32 = mybir.dt.float32
BF16 = mybir.dt.bfloat16
BF16NP = ml_dtypes.bfloat16

# Geometry (hardcoded per the problem spec)
B, D, H, W, Cin = 4, 32, 32, 32, 8
KD = KH = KW = 3
F = 16
OD = OH = OW = 30
L = OD * OH * OW           # 27000
P = KD * KH * KW * Cin     # 216
NCORE = 8
LC = L // NCORE            # 3375 locations per core
G = 32                     # locations per group
NG = 108                   # groups per core (LC padded to 3456)
LP = NG * G                # 3456
SG = 12                    # groups per DMA super-tile
NSUP = NG // SG            # 9
WG = 27                    # groups accumulated per output window (PSUM bank)
NW = NG // WG              # 4 windows per core
K1 = 128                   # contraction chunk 1 (p in [0,128))
K2 = 96                    # chunk 2: 88 kernel rows + bias row + 7 zero pad
NCOL = G * F               # 512 rhs columns per group
MROW = G * B               # 128 psum rows per group
RT1W = SG * (NCOL + MROW)  # 2560 cols: [r block | t block]
DELAY = 2                  # groups of lag before emitting MM2

_CACHE = {}


def _build(reps=1):
    nc = bacc.Bacc("TRN2", target_bir_lowering=False, debug=False)

    rt1 = nc.dram_tensor("rt1", [NSUP, K1, RT1W], BF16, kind="ExternalInput")
    rt2 = nc.dram_tensor("rt2", [NSUP, K2, RT1W], BF16, kind="ExternalInput")
    mask = nc.dram_tensor("mask", [MROW, NCOL], F32, kind="ExternalInput")
    sel = nc.dram_tensor("sel", [MROW, WG * MROW], BF16, kind="ExternalInput")
    out = nc.dram_tensor("out", [NW, MROW, NCOL], BF16, kind="ExternalOutput")

    TOFF = SG * NCOL  # column where the t block starts

    with tile.TileContext(nc) as tc, ExitStack() as ctx:
        const_pool = ctx.enter_context(tc.tile_pool(name="const", bufs=1))
        sup_pool = ctx.enter_context(tc.tile_pool(name="sup", bufs=3))
        s_pool = ctx.enter_context(tc.tile_pool(name="s", bufs=4))
        stage_pool = ctx.enter_context(tc.tile_pool(name="stage", bufs=2))
        ps1_pool = ctx.enter_context(tc.tile_pool(name="ps1", bufs=4, space="PSUM"))
        ps2_pool = ctx.enter_context(tc.tile_pool(name="ps2", bufs=2, space="PSUM"))

        mask_sb = const_pool.tile([MROW, NCOL], F32)
        nc.sync.dma_start(mask_sb[:], mask.ap())
        sel_sb = const_pool.tile([MROW, WG * MROW], BF16)
        nc.sync.dma_start(sel_sb[:], sel.ap())

        sup = {}
        state = {"psum2": None}

        def emit_mm2(g_rep, s_sb):
            g = g_rep % NG
            w, gh = g // WG, g % WG
            if gh == 0:
                state["psum2"] = ps2_pool.tile([MROW, NCOL], F32, name="psum2")
            psum2 = state["psum2"]
            nc.tensor.matmul(
                psum2[:],
                sel_sb[:, gh * MROW:(gh + 1) * MROW],
                s_sb[:],
                start=(gh == 0), stop=(gh == WG - 1),
                skip_group_check=True,
            )
            if gh == WG - 1:
                stage = stage_pool.tile([MROW, NCOL], BF16)
                nc.vector.tensor_copy(stage[:], psum2[:])
                nc.sync.dma_start(out.ap()[w], stage[:])

        pending = deque()
        for g_rep in range(reps * NG):
            g = g_rep % NG
            s, j = g // SG, g % SG
            if j == 0:
                sup["rt1"] = sup_pool.tile([K1, RT1W], BF16, tag="rt1", name="rt1sb")
                nc.sync.dma_start(sup["rt1"][:], rt1.ap()[s])
                sup["rt2"] = sup_pool.tile([K2, RT1W], BF16, tag="rt2", name="rt2sb")
                nc.scalar.dma_start(sup["rt2"][:], rt2.ap()[s])

            psum1 = ps1_pool.tile([MROW, NCOL], F32)
            nc.tensor.matmul(
                psum1[:],
                sup["rt1"][:, TOFF + j * MROW:TOFF + (j + 1) * MROW],
                sup["rt1"][:, j * NCOL:(j + 1) * NCOL],
                start=True, stop=False,
            )
            nc.tensor.matmul(
                psum1[:],
                sup["rt2"][:, TOFF + j * MROW:TOFF + (j + 1) * MROW],
                sup["rt2"][:, j * NCOL:(j + 1) * NCOL],
                start=False, stop=True,
            )
            s_sb = s_pool.tile([MROW, NCOL], BF16)
            nc.vector.tensor_mul(s_sb[:], psum1[:], mask_sb[:])

            pending.append((g_rep, s_sb))
            if len(pending) > DELAY:
                emit_mm2(*pending.popleft())
        while pending:
            emit_mm2(*pending.popleft())

    nc.compile()
    return nc


def _prep_inputs(x, kernel, bias):
    """Pack full inputs into per-core tile-layout bf16 arrays."""
    x = np.ascontiguousarray(x, dtype=np.float32)
    kernel = np.ascontiguousarray(kernel, dtype=np.float32)
    bias = np.ascontiguousarray(bias, dtype=np.float32).reshape(L, F)

    # im2col: patches[b, l, p] with p=(kd,kh,kw,cin), matching the reference
    sw = np.lib.stride_tricks.sliding_window_view(x, (KD, KH, KW), axis=(1, 2, 3))
    patches = sw.transpose(0, 1, 2, 3, 5, 6, 7, 4).reshape(B, L, P)

    mask_np = np.zeros((MROW, NCOL), dtype=np.float32)
    for l in range(G):
        mask_np[B * l:B * l + B, F * l:F * l + F] = 1.0
    # sel[k=(l',b), 128*gh + m] = 1 iff m == 4*gh + b  (m < WG*B)
    sel_np = np.zeros((MROW, WG * MROW), dtype=BF16NP)
    for gh in range(WG):
        for b in range(B):
            sel_np[b::B, MROW * gh + B * gh + b] = 1.0

    in_maps = []
    for c in range(NCORE):
        lo = c * LC
        kp = np.zeros((LP, P, F), dtype=np.float32)
        kp[:LC] = kernel[lo:lo + LC]
        bp = np.zeros((LP, F), dtype=np.float32)
        bp[:LC] = bias[lo:lo + LC]
        ap_ = np.zeros((B, LP, P), dtype=np.float32)
        ap_[:, :LC] = patches[:, lo:lo + LC]

        # r block: R[s, p, j*NCOL + l*F + f] = kp[s*128 + j*32 + l, p, f]
        r = kp.reshape(NSUP, SG, G, P, F).transpose(0, 3, 1, 2, 4).reshape(
            NSUP, P, SG * NCOL)
        biasrow = bp.reshape(NSUP, 1, SG * NCOL)
        # t block: T[s, p, j*MROW + l*B + b] = patches[b, s*128+j*32+l, p]
        t = ap_.reshape(B, NSUP, SG, G, P).transpose(1, 4, 2, 3, 0).reshape(
            NSUP, P, SG * MROW)

        rt1 = np.concatenate([r[:, :K1], t[:, :K1]], axis=2).astype(BF16NP)

        r2 = np.zeros((NSUP, K2, SG * NCOL), dtype=np.float32)
        r2[:, :P - K1] = r[:, K1:]
        r2[:, P - K1] = biasrow[:, 0]
        t2 = np.zeros((NSUP, K2, SG * MROW), dtype=np.float32)
        t2[:, :P - K1] = t[:, K1:]
        t2[:, P - K1] = 1.0
        rt2 = np.concatenate([r2, t2], axis=2).astype(BF16NP)

        in_maps.append(dict(rt1=rt1, rt2=rt2, mask=mask_np, sel=sel_np))
    return in_maps


def _unpack_output(results):
    """results: list of per-core dicts with 'out' [NW, MROW, NCOL] fp32."""
    slabs = []
    for c in range(NCORE):
        o = results[c]["out"][:, :WG * B].astype(np.float32).reshape(
            NW, WG, B, G, F)
        o = o.transpose(2, 0, 1, 3, 4).reshape(B, LP, F)[:, :LC]
        slabs.append(o)
    full = np.concatenate(slabs, axis=1)          # (B, L, F)
    return np.ascontiguousarray(full.reshape(B, OD, OH, OW, F))


def kernel(x, kernel, bias):
    if "nc" not in _CACHE:
        _CACHE["nc"] = _build()
    nc = _CACHE["nc"]
    in_maps = _prep_inputs(x, kernel, bias)
    res = bass_utils.run_bass_kernel_spmd(
        nc, in_maps, core_ids=list(range(NCORE)))
    return _unpack_output(res.results)

